# revision 23
# baseline (speedup 1.0000x reference)
"""DeformConv2d (DCNv2) Trainium2 Bass kernel, v2.

Problem: N=4, C_IN=C_OUT=64, H=W=128, 3x3 taps, stride=1, pad=1, dil=1,
modulated deformable conv (torchvision semantics).

Sharding: 8 cores; core = (image n = core//2, row-half = core%2).
Each core computes out[n, :, i0:i0+64, :] from the full image x[n].

v2 design (vs v1 baseline):
  - bf16 "quad" image in DRAM: entry (y,x) holds the 2x2 pixel block
    (y..y+1, x..x+1) x 64ch as 256 bf16 values ordered (c, q) with
    q = yc*2+xc.  One 512B gather descriptor fetches all 4 bilinear
    corners of one (pixel, tap) sample -> half the descriptors and half
    the HBM bytes of v1.
  - index repack j-major -> 16-partition-wrapped via two PE transpose
    stages (v1 used a DRAM bounce with 256B descriptors: ~460us).
  - corner combine: one 2x-mode DVE multiply (weights broadcast over
    channels via stride-0 AP) + one tensor_reduce over the 4-corner
    inner axis.
  - conv: row-pair transposes ([128j, 128(i2,c)] -> [(i2,c), j]) and
    per-tap 128-contraction matmuls with block-diagonal duplicated
    weights -> 2 output pixels per PE column.
"""
import sys

_TRN_REPO = "/opt/trn_rl_repo"
if _TRN_REPO not in sys.path:
    sys.path.insert(0, _TRN_REPO)

import numpy as np
import ml_dtypes

import concourse.bass as bass
import concourse.bacc as bacc
import concourse.tile as tile
import concourse.mybir as mybir
from concourse.bass_utils import run_bass_kernel_spmd
from contextlib import ExitStack

F32 = mybir.dt.float32
BF16 = mybir.dt.bfloat16
I16 = mybir.dt.int16
ALU = mybir.AluOpType
BF = ml_dtypes.bfloat16

N, C, H, W = 4, 64, 128, 128
K2 = 9
PAD = 16                    # coordinate padding on each side
PH = H + 2 * PAD            # 160
PW = W + 2 * PAD            # 160
NQ = PH * PW                # 25600 quad entries
HI = 64                     # rows per core
R = 16                      # rows per gather block
NBLK = HI // R              # 4
RSUB = 8                    # rows per dma_gather call (HW caps 1024 descs)
NIDX = RSUB * W             # descriptors per dma_gather call
CLAMP = 11.0                # |floor(offset)| clamp (pad-region safe)
MAGIC = 12582912.0          # 1.5 * 2**23 for round-to-nearest-even
DMA_SCRATCH = 49152         # SWDGE descriptor carveout (ring = this/16)

_CACHED = {}


def build_nc():
    nc = bacc.Bacc(trn_type="TRN2", debug=False, num_swdge_queues=4,
                   dynamic_dma_scratch_size=DMA_SCRATCH)

    xq_d = nc.dram_tensor("xq", [NQ * 256], BF16, kind="ExternalInput")
    offj_d = nc.dram_tensor("offj", [128, 2 * K2 * HI], F32, kind="ExternalInput").ap()
    maskj_d = nc.dram_tensor("maskj", [128, K2 * HI], F32, kind="ExternalInput").ap()
    idxb_d = nc.dram_tensor("idxb", [128, K2 * HI * 8], F32, kind="ExternalInput").ap()
    wk2_d = nc.dram_tensor("wk2", [128, K2 * 128], BF16, kind="ExternalInput").ap()
    identf_d = nc.dram_tensor("identf", [128, 128], F32, kind="ExternalInput").ap()
    identb_d = nc.dram_tensor("identb", [128, 128], BF16, kind="ExternalInput").ap()
    out_d = nc.dram_tensor("out", [64, HI * W], F32, kind="ExternalOutput").ap()

    # gather source: quad entries of the padded image
    src_ap = bass.AP(xq_d, 0, [[256, NQ], [1, 256]])

    NM = K2 * HI            # 576 (k, i) pairs
    NFREE = NM * 8          # 4608 descriptors per 16-partition wrap

    with ExitStack() as ctx:
        tc = ctx.enter_context(tile.TileContext(nc))

        const = ctx.enter_context(tc.tile_pool(name="const", bufs=1))
        live = ctx.enter_context(tc.tile_pool(name="live", bufs=1))
        ph1 = ExitStack()
        work = ph1.enter_context(tc.tile_pool(name="work", bufs=1))
        ps1pool = ph1.enter_context(tc.tile_pool(name="ps1", bufs=2, space="PSUM"))
        ps2pool = ph1.enter_context(tc.tile_pool(name="ps2", bufs=2, space="PSUM"))

        offj = work.tile([128, 2 * K2 * HI], F32)
        nc.sync.dma_start(offj[:], offj_d)
        identf = const.tile([128, 128], F32)
        nc.sync.dma_start(identf[:], identf_d)
        maskj = work.tile([128, K2 * HI], F32)
        nc.sync.dma_start(maskj[:], maskj_d)
        idxb = work.tile([128, NFREE], F32)
        nc.sync.dma_start(idxb[:], idxb_d)
        identb = const.tile([128, 128], BF16)
        nc.sync.dma_start(identb[:], identb_d)
        wk2 = const.tile([128, K2 * 128], BF16)
        nc.sync.dma_start(wk2[:], wk2_d)

        # ---- Phase 1a: floor, then dyx (critical path to the gathers) ----
        flo = work.tile([128, 2 * K2 * HI], F32)
        nc.vector.tensor_scalar(flo[:], offj[:], MAGIC, MAGIC, ALU.add,
                                ALU.subtract)
        rup = work.tile([128, 2 * K2 * HI], F32)
        nc.vector.tensor_tensor(rup[:], flo[:], offj[:], ALU.is_gt)
        nc.vector.tensor_tensor(flo[:], flo[:], rup[:], ALU.subtract)
        nc.vector.tensor_scalar(flo[:], flo[:], -CLAMP, CLAMP, ALU.max, ALU.min)

        def kv(t):  # [128, (k, two, i)]
            return t[:].rearrange("p (k two i) -> p k two i", k=K2, two=2, i=HI)

        # dyx[j, m=(k,i)] = floor(dy)*PW + floor(dx)
        dyx = work.tile([128, NM], F32)
        dyx3 = dyx[:].rearrange("p (k i) -> p k i", k=K2, i=HI)
        nc.vector.scalar_tensor_tensor(
            dyx3, kv(flo)[:, :, 0, :], float(PW), kv(flo)[:, :, 1, :],
            ALU.mult, ALU.add)

        # ---- Phase 1b: repack dyx [j, m] -> dyx2 [u, (t, jw, m_local)] --
        # stage 1: dyxT[m_local, t, j] via 5 PE transposes of [128, <=128]
        dyxT = work.tile([128, 5 * 128], F32)
        nc.vector.memset(dyxT[:], 0.0)
        for t in range(5):
            wdt = 128 if t < 4 else 64
            ps1 = ps1pool.tile([128, 128], F32)
            nc.tensor.transpose(
                ps1[0:wdt, :], dyx[:, t * 128:t * 128 + wdt], identf[:])
            nc.scalar.copy(dyxT[0:wdt, t * 128:(t + 1) * 128], ps1[0:wdt, :])
        # stage 2: dyx2[u, (t, jw, local)] via 40 transposes of [128, 16]
        dyx2 = work.tile([128, 5 * 8 * 128], F32)
        for t in range(5):
            ps2 = ps2pool.tile([128, 8 * 128], F32)
            for jw in range(8):
                nc.tensor.transpose(
                    ps2[0:16, jw * 128:(jw + 1) * 128],
                    dyxT[:, t * 128 + 16 * jw:t * 128 + 16 * jw + 16],
                    identf[:])
            nc.scalar.copy(dyx2[0:16, t * 1024:(t + 1) * 1024], ps2[0:16, :])

        # replicate dyx2 partitions 0-15 across all 8 groups (log tree),
        # then add the (host-replicated) base on all 128 partitions.
        nc.sync.dma_start(dyx2[16:32, :], dyx2[0:16, :])
        nc.sync.dma_start(dyx2[32:64, :], dyx2[0:32, :])
        nc.sync.dma_start(dyx2[64:128, :], dyx2[0:64, :])

        frac = work.tile([128, 2 * K2 * HI], F32)
        nc.vector.tensor_tensor(frac[:], offj[:], flo[:], ALU.subtract)

        # idxs[u, m*8 + jw] = idxb + dyx2  (int16)
        # iterate (t, local, jw): idxs/idxb at t*1024 + local*8 + jw,
        # dyx2 at t*1024 + jw*128 + local
        idxs = live.tile([128, NFREE], I16)
        nc.vector.tensor_tensor(
            idxs[:, 0:4096].rearrange("p (t l j) -> p t l j",
                                      t=4, l=128, j=8),
            idxb[:, 0:4096].rearrange("p (t l j) -> p t l j",
                                      t=4, l=128, j=8),
            dyx2[:, 0:4096].rearrange("p (t j l) -> p t l j",
                                      t=4, j=8, l=128),
            ALU.add)
        nc.vector.tensor_tensor(
            idxs[:, 4096:4608].rearrange("p (l j) -> p l j", l=64, j=8),
            idxb[:, 4096:4608].rearrange("p (l j) -> p l j", l=64, j=8),
            dyx2[:, 4096:5120].rearrange("p (j l) -> p l j",
                                         j=8, l=128)[:, 0:64, :],
            ALU.add)

        # ---- Phase 1c: corner weights w4[j, (k, i, q)] bf16, mask folded
        wy = kv(frac)[:, :, 0, :]      # [128, k, i]
        wx = kv(frac)[:, :, 1, :]
        omy = work.tile([128, NM], F32)
        omyv = omy[:].rearrange("p (k i) -> p k i", k=K2, i=HI)
        nc.vector.tensor_scalar(omyv, wy, 1.0, -1.0, ALU.subtract, ALU.mult)
        omx = work.tile([128, NM], F32)
        omxv = omx[:].rearrange("p (k i) -> p k i", k=K2, i=HI)
        nc.vector.tensor_scalar(omxv, wx, 1.0, -1.0, ALU.subtract, ALU.mult)
        m3 = maskj[:].rearrange("p (k i) -> p k i", k=K2, i=HI)
        wxm0 = work.tile([128, NM], F32)
        wxm0v = wxm0[:].rearrange("p (k i) -> p k i", k=K2, i=HI)
        nc.vector.tensor_tensor(wxm0v, omxv, m3, ALU.mult)
        wxm1 = work.tile([128, NM], F32)
        wxm1v = wxm1[:].rearrange("p (k i) -> p k i", k=K2, i=HI)
        nc.vector.tensor_tensor(wxm1v, wx, m3, ALU.mult)

        w4 = live.tile([128, NM * 4], BF16)
        w4v = w4[:].rearrange("p (k i q) -> p k i q", k=K2, i=HI, q=4)
        nc.vector.tensor_tensor(w4v[:, :, :, 0], omyv, wxm0v, ALU.mult)
        nc.vector.tensor_tensor(w4v[:, :, :, 1], omyv, wxm1v, ALU.mult)
        nc.vector.tensor_tensor(w4v[:, :, :, 2], wy, wxm0v, ALU.mult)
        nc.vector.tensor_tensor(w4v[:, :, :, 3], wy, wxm1v, ALU.mult)

        # ---- Phase 2: gather / combine / transpose / conv ----------------
        ph1.close()
        gpool = ctx.enter_context(tc.tile_pool(name="g", bufs=4))
        p4pool = ctx.enter_context(tc.tile_pool(name="p4", bufs=2))
        s2pool = ctx.enter_context(tc.tile_pool(name="s2", bufs=2))
        stpool = ctx.enter_context(tc.tile_pool(name="st", bufs=2))
        obpool = ctx.enter_context(tc.tile_pool(name="ob", bufs=2))
        tpps = ctx.enter_context(tc.tile_pool(name="tp", bufs=2, space="PSUM"))
        outps = ctx.enter_context(tc.tile_pool(name="ops", bufs=2, space="PSUM"))

        idxs4 = idxs[:].rearrange("p (k i jw) -> p k i jw", k=K2, i=HI, jw=8)
        w4r = w4[:].rearrange("p (k i q) -> p k i q", k=K2, i=HI, q=4)

        with nc.allow_low_precision("bf16 deformable-conv pipeline"):
            for b in range(NBLK):
                out_ps = outps.tile([128, R * 64], F32)
                for k in range(K2):
                    g = gpool.tile([128, R * 256], BF16)
                    gv = g[:].rearrange("p (s e) -> p s e", s=R, e=256)
                    for sub in range(R // RSUB):
                        nc.gpsimd.dma_gather(
                            gv[:, sub * RSUB:(sub + 1) * RSUB, :], src_ap,
                            idxs4[:, k,
                                  b * R + sub * RSUB:b * R + (sub + 1) * RSUB,
                                  :],
                            NIDX, NIDX, elem_size=256,
                            queue_num=(b * K2 * (R // RSUB) + k * (R // RSUB)
                                       + sub) % 4,
                        )
                    # weighted corners: p4 = g * w (w broadcast over c)
                    p4 = p4pool.tile([128, R * 256], BF16)
                    wsl = w4r[:, k, b * R:(b + 1) * R, :]
                    w_b = bass.AP(
                        wsl.tensor, wsl.offset,
                        [wsl.ap[0], [4, R], [0, C], [1, 4]],
                    )
                    nc.vector.tensor_tensor(
                        p4[:].rearrange("p (i c q) -> p i c q", i=R, c=C, q=4),
                        g[:].rearrange("p (i c q) -> p i c q", i=R, c=C, q=4),
                        w_b, ALU.mult)
                    # y-corner sum (pairwise: packed-pair reads keep DVE 2x);
                    # x-corner sum is folded into the matmul (two accumulating
                    # planes e=0/1 share the same block-diag stationary)
                    s2 = s2pool.tile([128, R * C * 2], BF16)
                    p4q = p4[:].rearrange("p (ic q2 e) -> p ic q2 e",
                                          ic=R * C, q2=2, e=2)
                    nc.vector.tensor_tensor(
                        s2[:].rearrange("p (ic e) -> p ic e", ic=R * C, e=2),
                        p4q[:, :, 0, :], p4q[:, :, 1, :], ALU.add)
                    # transpose row-pairs of each e-plane to [(i2, c), j]
                    tp = tpps.tile([128, 2 * 8 * 128], BF16)
                    s2v = s2[:].rearrange("p (h x c e) -> p h x c e",
                                          h=R // 2, x=2, c=C, e=2)
                    for e in range(2):
                        for h in range(R // 2):
                            nc.tensor.transpose(
                                tp[:, (e * 8 + h) * 128:(e * 8 + h + 1) * 128],
                                s2v[:, h, :, :, e], identb[:])
                    st = stpool.tile([128, 2 * 8 * 128], BF16)
                    nc.scalar.copy(st[:], tp[:])
                    for e in range(2):
                        for half in range(2):
                            nc.tensor.matmul(
                                out_ps[:, half * 512:(half + 1) * 512],
                                wk2[:, k * 128:(k + 1) * 128],
                                st[:, e * 1024 + half * 512:
                                   e * 1024 + (half + 1) * 512],
                                start=(k == 0 and e == 0),
                                stop=(k == K2 - 1 and e == 1))
                ob = obpool.tile([128, R * 64], F32)
                nc.scalar.copy(ob[:], out_ps[:])
                for i2 in range(2):
                    dst = bass.AP(
                        out_d.tensor, out_d.offset + (b * R + i2) * W,
                        [out_d.ap[0], [2 * W, R // 2], [1, W]],
                    )
                    nc.sync.dma_start(
                        dst,
                        ob[i2 * 64:(i2 + 1) * 64, :].rearrange(
                            "p (h j) -> p h j", h=R // 2, j=W))

    if not nc.is_finalized():
        nc.finalize()
    return nc


def _quad_image(xn):
    """xn: [C, H, W] f32 -> quad bf16 [NQ*256], entry (y,x) = 2x2 block,
    value order (c, q) with q = yc*2+xc."""
    xpad = np.zeros((PH + 1, PW + 1, C), dtype=BF)
    xpad[PAD:PAD + H, PAD:PAD + W, :] = xn.transpose(1, 2, 0).astype(BF)
    xq = np.empty((PH, PW, C, 4), dtype=BF)
    xq[:, :, :, 0] = xpad[0:PH, 0:PW]
    xq[:, :, :, 1] = xpad[0:PH, 1:PW + 1]
    xq[:, :, :, 2] = xpad[1:PH + 1, 0:PW]
    xq[:, :, :, 3] = xpad[1:PH + 1, 1:PW + 1]
    return np.ascontiguousarray(xq.reshape(-1))


def _static_prep(weight):
    # weight is [O, C_in, KH, KW]; reshape -> [O, C_in, K2]
    wk = weight.reshape(C, C, K2)
    wk2 = np.zeros((128, K2, 128), np.float32)
    for i2 in range(2):
        # rows (i2*64 + c), cols (i2*64 + o) = W[o, c, k]
        wk2[i2 * 64:(i2 + 1) * 64, :, i2 * 64:(i2 + 1) * 64] = (
            wk.transpose(1, 2, 0))
    return wk2.astype(BF).reshape(128, K2 * 128)


def _prep_core(x, offset, mask, wk2, xq_cache, core):
    n, half = core // 2, core % 2
    i0 = half * HI
    if n not in xq_cache:
        xq_cache[n] = _quad_image(x[n])
    offj = np.ascontiguousarray(
        offset[n, :, i0:i0 + HI, :].transpose(2, 0, 1)).reshape(128, 2 * K2 * HI)
    maskj = np.ascontiguousarray(
        mask[n, :, i0:i0 + HI, :].transpose(2, 0, 1)).reshape(128, K2 * HI)

    u = np.arange(128) % 16
    k = np.arange(K2)
    ki, kj = k // 3, k % 3
    i = np.arange(HI)
    jw = np.arange(8)
    # idxb[u, (k, i, jw)] = (PAD+i0+i+ki-1)*PW + PAD + jw*16 + u + kj - 1
    base = ((PAD + i0 + i[None, :, None] + ki[:, None, None] - 1) * PW
            + PAD + jw[None, None, :] * 16 + kj[:, None, None] - 1)  # [k, i, jw]
    idxb = (base[None] + u[:, None, None, None]).reshape(128, -1)
    assert idxb.min() - CLAMP * PW - CLAMP >= 0
    assert idxb.max() + CLAMP * PW + CLAMP < NQ

    return {
        "xq": xq_cache[n],
        "offj": offj,
        "maskj": maskj,
        "idxb": idxb.astype(np.float32),
        "wk2": wk2,
        "identf": np.eye(128, dtype=np.float32),
        "identb": np.eye(128, dtype=BF),
    }


def _prep_all(x, offset, mask, weight):
    x = np.asarray(x, np.float32)
    offset = np.asarray(offset, np.float32)
    mask = np.asarray(mask, np.float32)
    weight = np.asarray(weight, np.float32)
    wk2 = _static_prep(weight)
    xq_cache = {}
    return [
        _prep_core(x, offset, mask, wk2, xq_cache, core) for core in range(8)
    ]


def _collect(res):
    out = np.empty((N, C, H, W), np.float32)
    for core in range(8):
        n, half = core // 2, core % 2
        out[n, :, half * HI:(half + 1) * HI, :] = (
            res.results[core]["out"].reshape(C, HI, W))
    return out


def kernel_traced(x, offset, mask, weight, trace=True, trace_kwargs=None):
    """Like kernel() but with NTFF tracing; returns (out, BassKernelResults)."""
    if "nc" not in _CACHED:
        _CACHED["nc"] = build_nc()
    in_maps = _prep_all(x, offset, mask, weight)
    res = run_bass_kernel_spmd(_CACHED["nc"], in_maps, list(range(8)),
                               trace=trace, **(trace_kwargs or {}))
    return _collect(res), res


def kernel(x, offset, mask, weight):
    if "nc" not in _CACHED:
        _CACHED["nc"] = build_nc()
    in_maps = _prep_all(x, offset, mask, weight)
    res = run_bass_kernel_spmd(_CACHED["nc"], in_maps, list(range(8)))
    return _collect(res)


# revision 48
# speedup vs baseline: 1.2645x; 1.2645x over previous
"""DeformConv2d (DCNv2) Trainium2 Bass kernel, v2.

Problem: N=4, C_IN=C_OUT=64, H=W=128, 3x3 taps, stride=1, pad=1, dil=1,
modulated deformable conv (torchvision semantics).

Sharding: 8 cores; core = (image n = core//2, row-half = core%2).
Each core computes out[n, :, i0:i0+64, :] from the full image x[n].

v2 design (vs v1 baseline, ~1.06ms -> ~0.24ms):
  - bf16 "quad" image in DRAM: entry (y,x) holds the 2x2 pixel block
    (y..y+1, x..x+1) x 64ch as 256 bf16 values ordered (c, q) with
    q = yc*2+xc.  One 512B gather descriptor fetches all 4 bilinear
    corners of one (pixel, tap) sample -> half the descriptors and half
    the HBM bytes of v1.  (SWDGE gather ucode caps at 1024 descriptors
    per call - probed on HW; 1536+ hangs.)
  - index repack j-major -> 16-partition-wrapped+replicated via 8
    selection matmuls (static 0/1 stationaries E_jw), ACT psum->sbuf
    copies, and DVE adds (v1 used a DRAM bounce with 256B descriptors:
    ~460us serial).
  - corner combine: one 2x-mode DVE multiply (weights broadcast over
    channels via stride-0 free AP, corner axis innermost keeps
    packed-pair reads) + one pairwise y-corner add; the x-corner sum is
    folded into the conv matmul (e=0/1 planes accumulate in PSUM with
    the same stationary).
  - conv: row-pair transposes ([128j, 128(i2,c)] -> [(i2,c), j]) and
    per-tap 128-contraction matmuls with block-diagonal duplicated
    weights -> 2 output pixels per PE column, moving operand bf16.
  - phase-2 is gpsimd-bound: 72 gather calls x ~2.5us descriptor-gen.
    4 SWDGE queues + 4-deep gather tiles keep the DMA drain (37.8 MB at
    ~230 GB/s) fully overlapped.
"""
import sys

_TRN_REPO = "/opt/trn_rl_repo"
if _TRN_REPO not in sys.path:
    sys.path.insert(0, _TRN_REPO)

import numpy as np
import ml_dtypes

import concourse.bass as bass
import concourse.bacc as bacc
import concourse.tile as tile
import concourse.mybir as mybir
from concourse.bass_utils import run_bass_kernel_spmd
from contextlib import ExitStack

F32 = mybir.dt.float32
BF16 = mybir.dt.bfloat16
I16 = mybir.dt.int16
ALU = mybir.AluOpType
BF = ml_dtypes.bfloat16

N, C, H, W = 4, 64, 128, 128
K2 = 9
PAD = 16                    # coordinate padding on each side
PH = H + 2 * PAD            # 160
PW = W + 2 * PAD            # 160
NQ = PH * PW                # 25600 quad entries
HI = 64                     # rows per core
R = 16                      # rows per gather block
NBLK = HI // R              # 4
RSUB = 8                    # rows per dma_gather call (HW caps 1024 descs)
NIDX = RSUB * W             # descriptors per dma_gather call
CLAMP = 11.0                # |floor(offset)| clamp (pad-region safe)
MAGIC = 12582912.0          # 1.5 * 2**23 for round-to-nearest-even
DMA_SCRATCH = 49152         # SWDGE descriptor carveout (ring = this/16)

_CACHED = {}


def build_nc():
    nc = bacc.Bacc(trn_type="TRN2", debug=False, num_swdge_queues=4,
                   dynamic_dma_scratch_size=DMA_SCRATCH)

    xq_d = nc.dram_tensor("xq", [NQ * 256], BF16, kind="ExternalInput")
    offj_d = nc.dram_tensor("offj", [128, 2 * K2 * HI], F32, kind="ExternalInput").ap()
    maskj_d = nc.dram_tensor("maskj", [128, K2 * HI], F32, kind="ExternalInput").ap()
    idxb_d = nc.dram_tensor("idxb", [128, K2 * HI * 8], F32, kind="ExternalInput").ap()
    wk2_d = nc.dram_tensor("wk2", [128, K2 * 128], BF16, kind="ExternalInput").ap()
    ejw_d = nc.dram_tensor("ejw", [128, 8 * 128], F32, kind="ExternalInput").ap()
    identb_d = nc.dram_tensor("identb", [128, 128], BF16, kind="ExternalInput").ap()
    out_d = nc.dram_tensor("out", [64, HI * W], F32, kind="ExternalOutput").ap()

    # gather source: quad entries of the padded image
    src_ap = bass.AP(xq_d, 0, [[256, NQ], [1, 256]])

    NM = K2 * HI            # 576 (k, i) pairs
    NFREE = NM * 8          # 4608 descriptors per 16-partition wrap

    with ExitStack() as ctx:
        tc = ctx.enter_context(tile.TileContext(nc))

        const = ctx.enter_context(tc.tile_pool(name="const", bufs=1))
        live = ctx.enter_context(tc.tile_pool(name="live", bufs=1))
        ph1 = ExitStack()
        work = ph1.enter_context(tc.tile_pool(name="work", bufs=1))
        ps1pool = ph1.enter_context(tc.tile_pool(name="ps1", bufs=2, space="PSUM"))

        offj = work.tile([128, 2 * K2 * HI], F32)
        nc.sync.dma_start(offj[:], offj_d)
        ejw = const.tile([128, 8 * 128], F32)
        nc.sync.dma_start(ejw[:], ejw_d)
        maskj = work.tile([128, K2 * HI], F32)
        nc.sync.dma_start(maskj[:], maskj_d)
        idxb = work.tile([128, NFREE], F32)
        nc.sync.dma_start(idxb[:], idxb_d)
        identb = const.tile([128, 128], BF16)
        nc.sync.dma_start(identb[:], identb_d)
        wk2 = const.tile([128, K2 * 128], BF16)
        nc.sync.dma_start(wk2[:], wk2_d)

        # warm the SWDGE gather ucode lib + first-call overhead while the
        # inputs load (gathers quad 0 = zeros; result unused)
        dummy_idx = const.tile([128, 1], I16)
        nc.gpsimd.memset(dummy_idx[:], 0)
        dummy_g = work.tile([128, 256], BF16)
        for q in range(4):
            nc.gpsimd.dma_gather(
                dummy_g[:].rearrange("p (s e) -> p s e", s=1, e=256), src_ap,
                dummy_idx[:], 16, 16, elem_size=256, queue_num=q)

        # ---- Phase 1a: floor, then dyx (critical path to the gathers) ----
        flo = work.tile([128, 2 * K2 * HI], F32)
        nc.vector.tensor_scalar(flo[:], offj[:], MAGIC, MAGIC, ALU.add,
                                ALU.subtract)
        rup = work.tile([128, 2 * K2 * HI], F32)
        nc.vector.tensor_tensor(rup[:], flo[:], offj[:], ALU.is_gt)
        nc.vector.tensor_tensor(flo[:], flo[:], rup[:], ALU.subtract)
        nc.vector.tensor_scalar(flo[:], flo[:], -CLAMP, CLAMP, ALU.max, ALU.min)

        def kv(t):  # [128, (k, two, i)]
            return t[:].rearrange("p (k two i) -> p k two i", k=K2, two=2, i=HI)

        # dyx[j, m=(k,i)] = floor(dy)*PW + floor(dx)
        dyx = work.tile([128, NM], F32)
        dyx3 = dyx[:].rearrange("p (k i) -> p k i", k=K2, i=HI)
        nc.vector.scalar_tensor_tensor(
            dyx3, kv(flo)[:, :, 0, :], float(PW), kv(flo)[:, :, 1, :],
            ALU.mult, ALU.add)

        # ---- Phase 1b: repack dyx [j, m] -> idxs [u-wrap, (m, jw)] -------
        # One selection matmul per jw: dp[p, m] = dyx[16*jw + p%16, m]
        # (stationary E_jw[j, p] = (j//16==jw) & (j%16==p%16)), then
        # idxs[p, m*8+jw] = idxb + dp with the add reading PSUM directly.
        frac = work.tile([128, 2 * K2 * HI], F32)
        nc.vector.tensor_tensor(frac[:], offj[:], flo[:], ALU.subtract)

        idxs = live.tile([128, NFREE], I16)
        idxs3 = idxs[:].rearrange("p (m jw) -> p m jw", m=NM, jw=8)
        idxb3 = idxb[:].rearrange("p (m jw) -> p m jw", m=NM, jw=8)
        dps = work.tile([128, 8 * NM], F32)
        for jp in range(4):            # two jw per PSUM tile
            dp = ps1pool.tile([128, 2 * NM], F32)
            for j2 in range(2):
                jw = jp * 2 + j2
                # chunk at psum-bank (512 f32) boundaries within the tile
                cuts = sorted({j2 * NM, (j2 + 1) * NM}
                              | {b for b in (512, 1024)
                                 if j2 * NM < b < (j2 + 1) * NM})
                for lo, hi in zip(cuts, cuts[1:]):
                    nc.tensor.matmul(
                        dp[:, lo:hi],
                        ejw[:, jw * 128:(jw + 1) * 128],
                        dyx[:, lo - j2 * NM:hi - j2 * NM],
                        start=True, stop=True)
            nc.scalar.copy(dps[:, jp * 2 * NM:(jp + 1) * 2 * NM], dp[:])
            nc.vector.tensor_tensor(
                idxs3[:, :, jp * 2:(jp + 1) * 2],
                idxb3[:, :, jp * 2:(jp + 1) * 2],
                dps[:, jp * 2 * NM:(jp + 1) * 2 * NM].rearrange(
                    "p (j2 m) -> p m j2", j2=2, m=NM),
                ALU.add)

        # ---- Phase 1c: corner weights w4[j, (k, i, q)] bf16, mask folded
        wy = kv(frac)[:, :, 0, :]      # [128, k, i]
        wx = kv(frac)[:, :, 1, :]
        omy = work.tile([128, NM], F32)
        omyv = omy[:].rearrange("p (k i) -> p k i", k=K2, i=HI)
        nc.vector.tensor_scalar(omyv, wy, 1.0, -1.0, ALU.subtract, ALU.mult)
        omx = work.tile([128, NM], F32)
        omxv = omx[:].rearrange("p (k i) -> p k i", k=K2, i=HI)
        nc.vector.tensor_scalar(omxv, wx, 1.0, -1.0, ALU.subtract, ALU.mult)
        m3 = maskj[:].rearrange("p (k i) -> p k i", k=K2, i=HI)
        wxm0 = work.tile([128, NM], F32)
        wxm0v = wxm0[:].rearrange("p (k i) -> p k i", k=K2, i=HI)
        nc.vector.tensor_tensor(wxm0v, omxv, m3, ALU.mult)
        wxm1 = work.tile([128, NM], F32)
        wxm1v = wxm1[:].rearrange("p (k i) -> p k i", k=K2, i=HI)
        nc.vector.tensor_tensor(wxm1v, wx, m3, ALU.mult)

        w4 = live.tile([128, NM * 4], BF16)
        w4v = w4[:].rearrange("p (k i q) -> p k i q", k=K2, i=HI, q=4)
        nc.vector.tensor_tensor(w4v[:, :, :, 0], omyv, wxm0v, ALU.mult)
        nc.vector.tensor_tensor(w4v[:, :, :, 1], omyv, wxm1v, ALU.mult)
        nc.vector.tensor_tensor(w4v[:, :, :, 2], wy, wxm0v, ALU.mult)
        nc.vector.tensor_tensor(w4v[:, :, :, 3], wy, wxm1v, ALU.mult)

        # ---- Phase 2: gather / combine / transpose / conv ----------------
        ph1.close()
        gpool = ctx.enter_context(tc.tile_pool(name="g", bufs=4))
        p4pool = ctx.enter_context(tc.tile_pool(name="p4", bufs=2))
        s2pool = ctx.enter_context(tc.tile_pool(name="s2", bufs=2))
        stpool = ctx.enter_context(tc.tile_pool(name="st", bufs=2))
        obpool = ctx.enter_context(tc.tile_pool(name="ob", bufs=2))
        tpps = ctx.enter_context(tc.tile_pool(name="tp", bufs=2, space="PSUM"))
        outps = ctx.enter_context(tc.tile_pool(name="ops", bufs=2, space="PSUM"))

        idxs4 = idxs[:].rearrange("p (k i jw) -> p k i jw", k=K2, i=HI, jw=8)
        w4r = w4[:].rearrange("p (k i q) -> p k i q", k=K2, i=HI, q=4)

        def idx_slice(k, r0, r1):
            return idxs4[:, k, r0:r1, :]

        with nc.allow_low_precision("bf16 deformable-conv pipeline"):
            for b in range(NBLK):
                out_ps = outps.tile([128, R * 64], F32)
                for k in range(K2):
                    g = gpool.tile([128, R * 256], BF16)
                    gv = g[:].rearrange("p (s e) -> p s e", s=R, e=256)
                    for sub in range(R // RSUB):
                        nc.gpsimd.dma_gather(
                            gv[:, sub * RSUB:(sub + 1) * RSUB, :], src_ap,
                            idx_slice(k, b * R + sub * RSUB,
                                      b * R + (sub + 1) * RSUB),
                            NIDX, NIDX, elem_size=256,
                            single_packet=False,
                            queue_num=(b * K2 * (R // RSUB) + k * (R // RSUB)
                                       + sub) % 4,
                        )
                    # weighted corners: p4 = g * w (w broadcast over c);
                    # y-corner sum pairwise (packed-pair reads keep DVE 2x);
                    # x-corner sum folds into the matmul (two accumulating
                    # planes e=0/1 share the same block-diag stationary).
                    # The very last (b, k) runs per 8-row half to shorten the
                    # post-last-gather tail.
                    p4 = p4pool.tile([128, R * 256], BF16)
                    s2 = s2pool.tile([128, R * C * 2], BF16)
                    tp = tpps.tile([128, 2 * 8 * 128], BF16)
                    st = stpool.tile([128, 2 * 8 * 128], BF16)
                    s2v = s2[:].rearrange("p (h x c e) -> p h x c e",
                                          h=R // 2, x=2, c=C, e=2)
                    last = (b == NBLK - 1 and k == K2 - 1)
                    for half in ((0, 1) if last else (None,)):
                        if half is None:
                            r0, r1 = 0, R
                        else:
                            r0, r1 = half * RSUB, (half + 1) * RSUB
                        nr = r1 - r0
                        wsl = w4r[:, k, b * R + r0:b * R + r1, :]
                        w_b = bass.AP(
                            wsl.tensor, wsl.offset,
                            [wsl.ap[0], [4, nr], [0, C], [1, 4]],
                        )
                        nc.vector.tensor_tensor(
                            p4[:, r0 * 256:r1 * 256].rearrange(
                                "p (i c q) -> p i c q", i=nr, c=C, q=4),
                            g[:, r0 * 256:r1 * 256].rearrange(
                                "p (i c q) -> p i c q", i=nr, c=C, q=4),
                            w_b, ALU.mult)
                        p4q = p4[:, r0 * 256:r1 * 256].rearrange(
                            "p (ic q2 e) -> p ic q2 e", ic=nr * C, q2=2, e=2)
                        nc.vector.tensor_tensor(
                            s2[:, r0 * 128:r1 * 128].rearrange(
                                "p (ic e) -> p ic e", ic=nr * C, e=2),
                            p4q[:, :, 0, :], p4q[:, :, 1, :], ALU.add)
                        for e in range(2):
                            for h in range(r0 // 2, r1 // 2):
                                nc.tensor.transpose(
                                    tp[:, (e * 8 + h) * 128:
                                       (e * 8 + h + 1) * 128],
                                    s2v[:, h, :, :, e], identb[:])
                        nc.scalar.copy(
                            st[:].rearrange("p (e t) -> p e t",
                                            e=2, t=1024)[:, :, r0 * 64:r1 * 64],
                            tp[:].rearrange("p (e t) -> p e t",
                                            e=2, t=1024)[:, :, r0 * 64:r1 * 64])
                        for e in range(2):
                            for hh in range(r0 // 8, r1 // 8):
                                nc.tensor.matmul(
                                    out_ps[:, hh * 512:(hh + 1) * 512],
                                    wk2[:, k * 128:(k + 1) * 128],
                                    st[:, e * 1024 + hh * 512:
                                       e * 1024 + (hh + 1) * 512],
                                    start=(k == 0 and e == 0),
                                    stop=(k == K2 - 1 and e == 1))
                ob = obpool.tile([128, R * 64], F32)
                for half in ((0, 1) if b == NBLK - 1 else (None,)):
                    c0, c1 = (0, 1024) if half is None else (half * 512,
                                                            (half + 1) * 512)
                    nc.scalar.copy(ob[:, c0:c1], out_ps[:, c0:c1])
                    for i2 in range(2):
                        dst = bass.AP(
                            out_d.tensor,
                            out_d.offset + (b * R + c0 // 64 + i2) * W,
                            [out_d.ap[0], [2 * W, (c1 - c0) // 128], [1, W]],
                        )
                        nc.sync.dma_start(
                            dst,
                            ob[i2 * 64:(i2 + 1) * 64, c0:c1].rearrange(
                                "p (h j) -> p h j",
                                h=(c1 - c0) // 128, j=W))

    if not nc.is_finalized():
        nc.finalize()
    return nc


def _quad_image(xn):
    """xn: [C, H, W] f32 -> quad bf16 [NQ*256], entry (y,x) = 2x2 block,
    value order (c, q) with q = yc*2+xc."""
    xpad = np.zeros((PH + 1, PW + 1, C), dtype=BF)
    xpad[PAD:PAD + H, PAD:PAD + W, :] = xn.transpose(1, 2, 0).astype(BF)
    xq = np.empty((PH, PW, C, 4), dtype=BF)
    xq[:, :, :, 0] = xpad[0:PH, 0:PW]
    xq[:, :, :, 1] = xpad[0:PH, 1:PW + 1]
    xq[:, :, :, 2] = xpad[1:PH + 1, 0:PW]
    xq[:, :, :, 3] = xpad[1:PH + 1, 1:PW + 1]
    return np.ascontiguousarray(xq.reshape(-1))


def _static_prep(weight):
    # weight is [O, C_in, KH, KW]; reshape -> [O, C_in, K2]
    wk = weight.reshape(C, C, K2)
    wk2 = np.zeros((128, K2, 128), np.float32)
    for i2 in range(2):
        # rows (i2*64 + c), cols (i2*64 + o) = W[o, c, k]
        wk2[i2 * 64:(i2 + 1) * 64, :, i2 * 64:(i2 + 1) * 64] = (
            wk.transpose(1, 2, 0))
    return wk2.astype(BF).reshape(128, K2 * 128)


def _prep_core(x, offset, mask, wk2, xq_cache, core):
    n, half = core // 2, core % 2
    i0 = half * HI
    if n not in xq_cache:
        xq_cache[n] = _quad_image(x[n])
    offj = np.ascontiguousarray(
        offset[n, :, i0:i0 + HI, :].transpose(2, 0, 1)).reshape(128, 2 * K2 * HI)
    maskj = np.ascontiguousarray(
        mask[n, :, i0:i0 + HI, :].transpose(2, 0, 1)).reshape(128, K2 * HI)

    u = np.arange(128) % 16
    k = np.arange(K2)
    ki, kj = k // 3, k % 3
    i = np.arange(HI)
    jw = np.arange(8)
    # idxb[u, (k, i, jw)] = (PAD+i0+i+ki-1)*PW + PAD + jw*16 + u + kj - 1
    base = ((PAD + i0 + i[None, :, None] + ki[:, None, None] - 1) * PW
            + PAD + jw[None, None, :] * 16 + kj[:, None, None] - 1)  # [k, i, jw]
    idxb = (base[None] + u[:, None, None, None]).reshape(128, -1)
    assert idxb.min() - CLAMP * PW - CLAMP >= 0
    assert idxb.max() + CLAMP * PW + CLAMP < NQ

    jj = np.arange(128)
    pp = np.arange(128)
    ejw = ((jj[:, None] % 16 == pp[None, :] % 16)[:, None, :]
           & (jj[:, None, None] // 16 == np.arange(8)[None, :, None])
           ).astype(np.float32).reshape(128, 8 * 128)

    return {
        "xq": xq_cache[n],
        "offj": offj,
        "maskj": maskj,
        "idxb": idxb.astype(np.float32),
        "wk2": wk2,
        "ejw": ejw,
        "identb": np.eye(128, dtype=BF),
    }


def _prep_all(x, offset, mask, weight):
    x = np.asarray(x, np.float32)
    offset = np.asarray(offset, np.float32)
    mask = np.asarray(mask, np.float32)
    weight = np.asarray(weight, np.float32)
    wk2 = _static_prep(weight)
    xq_cache = {}
    return [
        _prep_core(x, offset, mask, wk2, xq_cache, core) for core in range(8)
    ]


def _collect(res):
    out = np.empty((N, C, H, W), np.float32)
    for core in range(8):
        n, half = core // 2, core % 2
        out[n, :, half * HI:(half + 1) * HI, :] = (
            res.results[core]["out"].reshape(C, HI, W))
    return out


def kernel_traced(x, offset, mask, weight, trace=True, trace_kwargs=None):
    """Like kernel() but with NTFF tracing; returns (out, BassKernelResults)."""
    if "nc" not in _CACHED:
        _CACHED["nc"] = build_nc()
    in_maps = _prep_all(x, offset, mask, weight)
    res = run_bass_kernel_spmd(_CACHED["nc"], in_maps, list(range(8)),
                               trace=trace, **(trace_kwargs or {}))
    return _collect(res), res


def kernel(x, offset, mask, weight):
    if "nc" not in _CACHED:
        _CACHED["nc"] = build_nc()
    in_maps = _prep_all(x, offset, mask, weight)
    res = run_bass_kernel_spmd(_CACHED["nc"], in_maps, list(range(8)))
    return _collect(res)


# revision 51
# speedup vs baseline: 1.2908x; 1.0208x over previous
"""DeformConv2d (DCNv2) Trainium2 Bass kernel, v2.

Problem: N=4, C_IN=C_OUT=64, H=W=128, 3x3 taps, stride=1, pad=1, dil=1,
modulated deformable conv (torchvision semantics).

Sharding: 8 cores; core = (image n = core//2, row-half = core%2).
Each core computes out[n, :, i0:i0+64, :] from the full image x[n].

v2 design (vs v1 baseline, ~1.06ms -> ~0.24ms):
  - bf16 "quad" image in DRAM: entry (y,x) holds the 2x2 pixel block
    (y..y+1, x..x+1) x 64ch as 256 bf16 values ordered (c, q) with
    q = yc*2+xc.  One 512B gather descriptor fetches all 4 bilinear
    corners of one (pixel, tap) sample -> half the descriptors and half
    the HBM bytes of v1.  (SWDGE gather ucode caps at 1024 descriptors
    per call - probed on HW; 1536+ hangs.)
  - index repack j-major -> 16-partition-wrapped+replicated via 8
    selection matmuls (static 0/1 stationaries E_jw), ACT psum->sbuf
    copies, and DVE adds (v1 used a DRAM bounce with 256B descriptors:
    ~460us serial).
  - corner combine: one 2x-mode DVE multiply (weights broadcast over
    channels via stride-0 free AP, corner axis innermost keeps
    packed-pair reads) + one pairwise y-corner add; the x-corner sum is
    folded into the conv matmul (e=0/1 planes accumulate in PSUM with
    the same stationary).
  - conv: row-pair transposes ([128j, 128(i2,c)] -> [(i2,c), j]) and
    per-tap 128-contraction matmuls with block-diagonal duplicated
    weights -> 2 output pixels per PE column, moving operand bf16.
  - phase-2 is gpsimd-bound: 72 gather calls x ~2.5us descriptor-gen.
    4 SWDGE queues + 4-deep gather tiles keep the DMA drain (37.8 MB at
    ~230 GB/s) fully overlapped.
"""
import sys

_TRN_REPO = "/opt/trn_rl_repo"
if _TRN_REPO not in sys.path:
    sys.path.insert(0, _TRN_REPO)

import numpy as np
import ml_dtypes

import concourse.bass as bass
import concourse.bacc as bacc
import concourse.tile as tile
import concourse.mybir as mybir
from concourse.bass_utils import run_bass_kernel_spmd
from contextlib import ExitStack

F32 = mybir.dt.float32
BF16 = mybir.dt.bfloat16
I16 = mybir.dt.int16
ALU = mybir.AluOpType
BF = ml_dtypes.bfloat16

N, C, H, W = 4, 64, 128, 128
K2 = 9
PAD = 16                    # coordinate padding on each side
PH = H + 2 * PAD            # 160
PW = W + 2 * PAD            # 160
NQ = PH * PW                # 25600 quad entries
HI = 64                     # rows per core
R = 16                      # rows per gather block
NBLK = HI // R              # 4
RSUB = 8                    # rows per dma_gather call (HW caps 1024 descs)
NIDX = RSUB * W             # descriptors per dma_gather call
CLAMP = 11.0                # |floor(offset)| clamp (pad-region safe)
MAGIC = 12582912.0          # 1.5 * 2**23 for round-to-nearest-even
DMA_SCRATCH = 49152         # SWDGE descriptor carveout (ring = this/16)

_CACHED = {}


def build_nc():
    nc = bacc.Bacc(trn_type="TRN2", debug=False, num_swdge_queues=4,
                   dynamic_dma_scratch_size=DMA_SCRATCH)

    xq_d = nc.dram_tensor("xq", [NQ * 256], BF16, kind="ExternalInput")
    offj_d = nc.dram_tensor("offj", [128, 2 * K2 * HI], F32, kind="ExternalInput").ap()
    maskj_d = nc.dram_tensor("maskj", [128, K2 * HI], F32, kind="ExternalInput").ap()
    idxb_d = nc.dram_tensor("idxb", [128, K2 * HI * 8], F32, kind="ExternalInput").ap()
    wk2_d = nc.dram_tensor("wk2", [128, K2 * 128], BF16, kind="ExternalInput").ap()
    ejw_d = nc.dram_tensor("ejw", [128, 8 * 128], F32, kind="ExternalInput").ap()
    identb_d = nc.dram_tensor("identb", [128, 128], BF16, kind="ExternalInput").ap()
    out_d = nc.dram_tensor("out", [64, HI * W], F32, kind="ExternalOutput").ap()

    # gather source: quad entries of the padded image
    src_ap = bass.AP(xq_d, 0, [[256, NQ], [1, 256]])

    NM = K2 * HI            # 576 (k, i) pairs
    NFREE = NM * 8          # 4608 descriptors per 16-partition wrap

    with ExitStack() as ctx:
        tc = ctx.enter_context(tile.TileContext(nc))

        const = ctx.enter_context(tc.tile_pool(name="const", bufs=1))
        live = ctx.enter_context(tc.tile_pool(name="live", bufs=1))
        ph1 = ExitStack()
        work = ph1.enter_context(tc.tile_pool(name="work", bufs=1))
        ps1pool = ph1.enter_context(tc.tile_pool(name="ps1", bufs=2, space="PSUM"))

        offj = work.tile([128, 2 * K2 * HI], F32)
        nc.sync.dma_start(offj[:], offj_d)
        ejw = const.tile([128, 8 * 128], F32)
        nc.sync.dma_start(ejw[:], ejw_d)
        maskj = work.tile([128, K2 * HI], F32)
        nc.sync.dma_start(maskj[:], maskj_d)
        idxb = work.tile([128, NFREE], F32)
        nc.sync.dma_start(idxb[:], idxb_d)
        identb = const.tile([128, 128], BF16)
        nc.sync.dma_start(identb[:], identb_d)
        wk2 = const.tile([128, K2 * 128], BF16)
        nc.sync.dma_start(wk2[:], wk2_d)

        # warm the SWDGE gather ucode lib + first-call overhead while the
        # inputs load (gathers quad 0 = zeros; result unused)
        dummy_idx = const.tile([128, 1], I16)
        nc.gpsimd.memset(dummy_idx[:], 0)
        dummy_g = work.tile([128, 256], BF16)
        for q in range(4):
            nc.gpsimd.dma_gather(
                dummy_g[:].rearrange("p (s e) -> p s e", s=1, e=256), src_ap,
                dummy_idx[:], 16, 16, elem_size=256, queue_num=q)

        # ---- Phase 1a: floor, then dyx (critical path to the gathers) ----
        flo = work.tile([128, 2 * K2 * HI], F32)
        nc.vector.tensor_scalar(flo[:], offj[:], MAGIC, MAGIC, ALU.add,
                                ALU.subtract)
        rup = work.tile([128, 2 * K2 * HI], F32)
        nc.vector.tensor_tensor(rup[:], flo[:], offj[:], ALU.is_gt)
        nc.vector.tensor_tensor(flo[:], flo[:], rup[:], ALU.subtract)
        nc.vector.tensor_scalar(flo[:], flo[:], -CLAMP, CLAMP, ALU.max, ALU.min)

        def kv(t):  # [128, (k, two, i)]
            return t[:].rearrange("p (k two i) -> p k two i", k=K2, two=2, i=HI)

        # dyx[j, m=(k,i)] = floor(dy)*PW + floor(dx)
        dyx = work.tile([128, NM], F32)
        dyx3 = dyx[:].rearrange("p (k i) -> p k i", k=K2, i=HI)
        nc.vector.scalar_tensor_tensor(
            dyx3, kv(flo)[:, :, 0, :], float(PW), kv(flo)[:, :, 1, :],
            ALU.mult, ALU.add)

        # ---- Phase 1b: repack dyx [j, m] -> idxs [u-wrap, (m, jw)] -------
        # One selection matmul per jw: dp[p, m] = dyx[16*jw + p%16, m]
        # (stationary E_jw[j, p] = (j//16==jw) & (j%16==p%16)), then
        # idxs[p, m*8+jw] = idxb + dp with the add reading PSUM directly.
        frac = work.tile([128, 2 * K2 * HI], F32)
        nc.vector.tensor_tensor(frac[:], offj[:], flo[:], ALU.subtract)

        idxs = live.tile([128, NFREE], I16)
        idxs3 = idxs[:].rearrange("p (m jw) -> p m jw", m=NM, jw=8)
        idxb3 = idxb[:].rearrange("p (m jw) -> p m jw", m=NM, jw=8)
        dps = work.tile([128, 8 * NM], F32)
        for jp in range(4):            # two jw per PSUM tile
            dp = ps1pool.tile([128, 2 * NM], F32)
            for j2 in range(2):
                jw = jp * 2 + j2
                # chunk at psum-bank (512 f32) boundaries within the tile
                cuts = sorted({j2 * NM, (j2 + 1) * NM}
                              | {b for b in (512, 1024)
                                 if j2 * NM < b < (j2 + 1) * NM})
                for lo, hi in zip(cuts, cuts[1:]):
                    nc.tensor.matmul(
                        dp[:, lo:hi],
                        ejw[:, jw * 128:(jw + 1) * 128],
                        dyx[:, lo - j2 * NM:hi - j2 * NM],
                        start=True, stop=True)
            nc.scalar.copy(dps[:, jp * 2 * NM:(jp + 1) * 2 * NM], dp[:])
            nc.vector.tensor_tensor(
                idxs3[:, :, jp * 2:(jp + 1) * 2],
                idxb3[:, :, jp * 2:(jp + 1) * 2],
                dps[:, jp * 2 * NM:(jp + 1) * 2 * NM].rearrange(
                    "p (j2 m) -> p m j2", j2=2, m=NM),
                ALU.add)

        # ---- Phase 1c: corner weights w4[j, (k, i, q)] bf16, mask folded
        wy = kv(frac)[:, :, 0, :]      # [128, k, i]
        wx = kv(frac)[:, :, 1, :]
        omy = work.tile([128, NM], F32)
        omyv = omy[:].rearrange("p (k i) -> p k i", k=K2, i=HI)
        nc.vector.tensor_scalar(omyv, wy, 1.0, -1.0, ALU.subtract, ALU.mult)
        omx = work.tile([128, NM], F32)
        omxv = omx[:].rearrange("p (k i) -> p k i", k=K2, i=HI)
        nc.vector.tensor_scalar(omxv, wx, 1.0, -1.0, ALU.subtract, ALU.mult)
        m3 = maskj[:].rearrange("p (k i) -> p k i", k=K2, i=HI)
        wxm0 = work.tile([128, NM], F32)
        wxm0v = wxm0[:].rearrange("p (k i) -> p k i", k=K2, i=HI)
        nc.vector.tensor_tensor(wxm0v, omxv, m3, ALU.mult)
        wxm1 = work.tile([128, NM], F32)
        wxm1v = wxm1[:].rearrange("p (k i) -> p k i", k=K2, i=HI)
        nc.vector.tensor_tensor(wxm1v, wx, m3, ALU.mult)

        w4 = live.tile([128, NM * 4], BF16)
        w4v = w4[:].rearrange("p (k i q) -> p k i q", k=K2, i=HI, q=4)
        nc.vector.tensor_tensor(w4v[:, :, :, 0], omyv, wxm0v, ALU.mult)
        nc.vector.tensor_tensor(w4v[:, :, :, 1], omyv, wxm1v, ALU.mult)
        nc.vector.tensor_tensor(w4v[:, :, :, 2], wy, wxm0v, ALU.mult)
        nc.vector.tensor_tensor(w4v[:, :, :, 3], wy, wxm1v, ALU.mult)

        # ---- Phase 2: gather / combine / transpose / conv ----------------
        ph1.close()
        gpool = ctx.enter_context(tc.tile_pool(name="g", bufs=8))
        p4pool = ctx.enter_context(tc.tile_pool(name="p4", bufs=2))
        s2pool = ctx.enter_context(tc.tile_pool(name="s2", bufs=2))
        stpool = ctx.enter_context(tc.tile_pool(name="st", bufs=2))
        obpool = ctx.enter_context(tc.tile_pool(name="ob", bufs=2))
        tpps = ctx.enter_context(tc.tile_pool(name="tp", bufs=2, space="PSUM"))
        outps = ctx.enter_context(tc.tile_pool(name="ops", bufs=2, space="PSUM"))

        idxs4 = idxs[:].rearrange("p (k i jw) -> p k i jw", k=K2, i=HI, jw=8)
        w4r = w4[:].rearrange("p (k i q) -> p k i q", k=K2, i=HI, q=4)

        def idx_slice(k, r0, r1):
            return idxs4[:, k, r0:r1, :]

        with nc.allow_low_precision("bf16 deformable-conv pipeline"):
            for b in range(NBLK):
                out_ps = outps.tile([128, R * 64], F32)
                for k in range(K2):
                    g = gpool.tile([128, R * 256], BF16)
                    gv = g[:].rearrange("p (s e) -> p s e", s=R, e=256)
                    for sub in range(R // RSUB):
                        nc.gpsimd.dma_gather(
                            gv[:, sub * RSUB:(sub + 1) * RSUB, :], src_ap,
                            idx_slice(k, b * R + sub * RSUB,
                                      b * R + (sub + 1) * RSUB),
                            NIDX, NIDX, elem_size=256,
                            single_packet=False,
                            queue_num=(b * K2 * (R // RSUB) + k * (R // RSUB)
                                       + sub) % 4,
                        )
                    # weighted corners: p4 = g * w (w broadcast over c);
                    # y-corner sum pairwise (packed-pair reads keep DVE 2x);
                    # x-corner sum folds into the matmul (two accumulating
                    # planes e=0/1 share the same block-diag stationary).
                    # The very last (b, k) runs per 8-row half to shorten the
                    # post-last-gather tail.
                    p4 = p4pool.tile([128, R * 256], BF16)
                    s2 = s2pool.tile([128, R * C * 2], BF16)
                    tp = tpps.tile([128, 2 * 8 * 128], BF16)
                    st = stpool.tile([128, 2 * 8 * 128], BF16)
                    s2v = s2[:].rearrange("p (h x c e) -> p h x c e",
                                          h=R // 2, x=2, c=C, e=2)
                    for half in (0, 1):
                        if half is None:
                            r0, r1 = 0, R
                        else:
                            r0, r1 = half * RSUB, (half + 1) * RSUB
                        nr = r1 - r0
                        wsl = w4r[:, k, b * R + r0:b * R + r1, :]
                        w_b = bass.AP(
                            wsl.tensor, wsl.offset,
                            [wsl.ap[0], [4, nr], [0, C], [1, 4]],
                        )
                        nc.vector.tensor_tensor(
                            p4[:, r0 * 256:r1 * 256].rearrange(
                                "p (i c q) -> p i c q", i=nr, c=C, q=4),
                            g[:, r0 * 256:r1 * 256].rearrange(
                                "p (i c q) -> p i c q", i=nr, c=C, q=4),
                            w_b, ALU.mult)
                        p4q = p4[:, r0 * 256:r1 * 256].rearrange(
                            "p (ic q2 e) -> p ic q2 e", ic=nr * C, q2=2, e=2)
                        nc.vector.tensor_tensor(
                            s2[:, r0 * 128:r1 * 128].rearrange(
                                "p (ic e) -> p ic e", ic=nr * C, e=2),
                            p4q[:, :, 0, :], p4q[:, :, 1, :], ALU.add)
                        for e in range(2):
                            for h in range(r0 // 2, r1 // 2):
                                nc.tensor.transpose(
                                    tp[:, (e * 8 + h) * 128:
                                       (e * 8 + h + 1) * 128],
                                    s2v[:, h, :, :, e], identb[:])
                        nc.scalar.copy(
                            st[:].rearrange("p (e t) -> p e t",
                                            e=2, t=1024)[:, :, r0 * 64:r1 * 64],
                            tp[:].rearrange("p (e t) -> p e t",
                                            e=2, t=1024)[:, :, r0 * 64:r1 * 64])
                        for e in range(2):
                            for hh in range(r0 // 8, r1 // 8):
                                nc.tensor.matmul(
                                    out_ps[:, hh * 512:(hh + 1) * 512],
                                    wk2[:, k * 128:(k + 1) * 128],
                                    st[:, e * 1024 + hh * 512:
                                       e * 1024 + (hh + 1) * 512],
                                    start=(k == 0 and e == 0),
                                    stop=(k == K2 - 1 and e == 1))
                ob = obpool.tile([128, R * 64], F32)
                for half in (0, 1):
                    c0, c1 = (0, 1024) if half is None else (half * 512,
                                                            (half + 1) * 512)
                    nc.scalar.copy(ob[:, c0:c1], out_ps[:, c0:c1])
                    for i2 in range(2):
                        dst = bass.AP(
                            out_d.tensor,
                            out_d.offset + (b * R + c0 // 64 + i2) * W,
                            [out_d.ap[0], [2 * W, (c1 - c0) // 128], [1, W]],
                        )
                        nc.sync.dma_start(
                            dst,
                            ob[i2 * 64:(i2 + 1) * 64, c0:c1].rearrange(
                                "p (h j) -> p h j",
                                h=(c1 - c0) // 128, j=W))

    if not nc.is_finalized():
        nc.finalize()
    return nc


def _quad_image(xn):
    """xn: [C, H, W] f32 -> quad bf16 [NQ*256], entry (y,x) = 2x2 block,
    value order (c, q) with q = yc*2+xc."""
    xpad = np.zeros((PH + 1, PW + 1, C), dtype=BF)
    xpad[PAD:PAD + H, PAD:PAD + W, :] = xn.transpose(1, 2, 0).astype(BF)
    xq = np.empty((PH, PW, C, 4), dtype=BF)
    xq[:, :, :, 0] = xpad[0:PH, 0:PW]
    xq[:, :, :, 1] = xpad[0:PH, 1:PW + 1]
    xq[:, :, :, 2] = xpad[1:PH + 1, 0:PW]
    xq[:, :, :, 3] = xpad[1:PH + 1, 1:PW + 1]
    return np.ascontiguousarray(xq.reshape(-1))


def _static_prep(weight):
    # weight is [O, C_in, KH, KW]; reshape -> [O, C_in, K2]
    wk = weight.reshape(C, C, K2)
    wk2 = np.zeros((128, K2, 128), np.float32)
    for i2 in range(2):
        # rows (i2*64 + c), cols (i2*64 + o) = W[o, c, k]
        wk2[i2 * 64:(i2 + 1) * 64, :, i2 * 64:(i2 + 1) * 64] = (
            wk.transpose(1, 2, 0))
    return wk2.astype(BF).reshape(128, K2 * 128)


def _prep_core(x, offset, mask, wk2, xq_cache, core):
    n, half = core // 2, core % 2
    i0 = half * HI
    if n not in xq_cache:
        xq_cache[n] = _quad_image(x[n])
    offj = np.ascontiguousarray(
        offset[n, :, i0:i0 + HI, :].transpose(2, 0, 1)).reshape(128, 2 * K2 * HI)
    maskj = np.ascontiguousarray(
        mask[n, :, i0:i0 + HI, :].transpose(2, 0, 1)).reshape(128, K2 * HI)

    u = np.arange(128) % 16
    k = np.arange(K2)
    ki, kj = k // 3, k % 3
    i = np.arange(HI)
    jw = np.arange(8)
    # idxb[u, (k, i, jw)] = (PAD+i0+i+ki-1)*PW + PAD + jw*16 + u + kj - 1
    base = ((PAD + i0 + i[None, :, None] + ki[:, None, None] - 1) * PW
            + PAD + jw[None, None, :] * 16 + kj[:, None, None] - 1)  # [k, i, jw]
    idxb = (base[None] + u[:, None, None, None]).reshape(128, -1)
    assert idxb.min() - CLAMP * PW - CLAMP >= 0
    assert idxb.max() + CLAMP * PW + CLAMP < NQ

    jj = np.arange(128)
    pp = np.arange(128)
    ejw = ((jj[:, None] % 16 == pp[None, :] % 16)[:, None, :]
           & (jj[:, None, None] // 16 == np.arange(8)[None, :, None])
           ).astype(np.float32).reshape(128, 8 * 128)

    return {
        "xq": xq_cache[n],
        "offj": offj,
        "maskj": maskj,
        "idxb": idxb.astype(np.float32),
        "wk2": wk2,
        "ejw": ejw,
        "identb": np.eye(128, dtype=BF),
    }


def _prep_all(x, offset, mask, weight):
    x = np.asarray(x, np.float32)
    offset = np.asarray(offset, np.float32)
    mask = np.asarray(mask, np.float32)
    weight = np.asarray(weight, np.float32)
    wk2 = _static_prep(weight)
    xq_cache = {}
    return [
        _prep_core(x, offset, mask, wk2, xq_cache, core) for core in range(8)
    ]


def _collect(res):
    out = np.empty((N, C, H, W), np.float32)
    for core in range(8):
        n, half = core // 2, core % 2
        out[n, :, half * HI:(half + 1) * HI, :] = (
            res.results[core]["out"].reshape(C, HI, W))
    return out


def kernel_traced(x, offset, mask, weight, trace=True, trace_kwargs=None):
    """Like kernel() but with NTFF tracing; returns (out, BassKernelResults)."""
    if "nc" not in _CACHED:
        _CACHED["nc"] = build_nc()
    in_maps = _prep_all(x, offset, mask, weight)
    res = run_bass_kernel_spmd(_CACHED["nc"], in_maps, list(range(8)),
                               trace=trace, **(trace_kwargs or {}))
    return _collect(res), res


def kernel(x, offset, mask, weight):
    if "nc" not in _CACHED:
        _CACHED["nc"] = build_nc()
    in_maps = _prep_all(x, offset, mask, weight)
    res = run_bass_kernel_spmd(_CACHED["nc"], in_maps, list(range(8)))
    return _collect(res)


# revision 52
# speedup vs baseline: 1.3592x; 1.0530x over previous
"""DeformConv2d (DCNv2) Trainium2 Bass kernel, v2.

Problem: N=4, C_IN=C_OUT=64, H=W=128, 3x3 taps, stride=1, pad=1, dil=1,
modulated deformable conv (torchvision semantics).

Sharding: 8 cores; core = (image n = core//2, row-half = core%2).
Each core computes out[n, :, i0:i0+64, :] from the full image x[n].

v2 design (vs v1 baseline, ~1.06ms -> ~0.24ms):
  - bf16 "quad" image in DRAM: entry (y,x) holds the 2x2 pixel block
    (y..y+1, x..x+1) x 64ch as 256 bf16 values ordered (c, q) with
    q = yc*2+xc.  One 512B gather descriptor fetches all 4 bilinear
    corners of one (pixel, tap) sample -> half the descriptors and half
    the HBM bytes of v1.  (SWDGE gather ucode caps at 1024 descriptors
    per call - probed on HW; 1536+ hangs.)
  - index repack j-major -> 16-partition-wrapped+replicated via 8
    selection matmuls (static 0/1 stationaries E_jw), ACT psum->sbuf
    copies, and DVE adds (v1 used a DRAM bounce with 256B descriptors:
    ~460us serial).
  - corner combine: one 2x-mode DVE multiply (weights broadcast over
    channels via stride-0 free AP, corner axis innermost keeps
    packed-pair reads) + one pairwise y-corner add; the x-corner sum is
    folded into the conv matmul (e=0/1 planes accumulate in PSUM with
    the same stationary).
  - conv: row-pair transposes ([128j, 128(i2,c)] -> [(i2,c), j]) and
    per-tap 128-contraction matmuls with block-diagonal duplicated
    weights -> 2 output pixels per PE column, moving operand bf16.
  - phase-2 is gpsimd-bound: 72 gather calls x ~2.5us descriptor-gen.
    4 SWDGE queues + 4-deep gather tiles keep the DMA drain (37.8 MB at
    ~230 GB/s) fully overlapped.
"""
import sys

_TRN_REPO = "/opt/trn_rl_repo"
if _TRN_REPO not in sys.path:
    sys.path.insert(0, _TRN_REPO)

import numpy as np
import ml_dtypes

import concourse.bass as bass
import concourse.bacc as bacc
import concourse.tile as tile
import concourse.mybir as mybir
from concourse.bass_utils import run_bass_kernel_spmd
from contextlib import ExitStack

F32 = mybir.dt.float32
BF16 = mybir.dt.bfloat16
I16 = mybir.dt.int16
ALU = mybir.AluOpType
BF = ml_dtypes.bfloat16

N, C, H, W = 4, 64, 128, 128
K2 = 9
PAD = 16                    # coordinate padding on each side
PH = H + 2 * PAD            # 160
PW = W + 2 * PAD            # 160
NQ = PH * PW                # 25600 quad entries
HI = 64                     # rows per core
R = 16                      # rows per gather block
NBLK = HI // R              # 4
RSUB = 8                    # rows per dma_gather call (HW caps 1024 descs)
NIDX = RSUB * W             # descriptors per dma_gather call
CLAMP = 11.0                # |floor(offset)| clamp (pad-region safe)
MAGIC = 12582912.0          # 1.5 * 2**23 for round-to-nearest-even
DMA_SCRATCH = 49152         # SWDGE descriptor carveout (ring = this/16)

_CACHED = {}


def build_nc():
    nc = bacc.Bacc(trn_type="TRN2", debug=False, num_swdge_queues=4,
                   dynamic_dma_scratch_size=DMA_SCRATCH)

    xq_d = nc.dram_tensor("xq", [NQ * 256], BF16, kind="ExternalInput")
    offj_d = nc.dram_tensor("offj", [128, 2 * K2 * HI], F32, kind="ExternalInput").ap()
    maskj_d = nc.dram_tensor("maskj", [128, K2 * HI], F32, kind="ExternalInput").ap()
    idxb_d = nc.dram_tensor("idxb", [128, K2 * HI * 8], F32, kind="ExternalInput").ap()
    wk2_d = nc.dram_tensor("wk2", [128, K2 * 128], BF16, kind="ExternalInput").ap()
    ejw_d = nc.dram_tensor("ejw", [128, 8 * 128], F32, kind="ExternalInput").ap()
    identb_d = nc.dram_tensor("identb", [128, 128], BF16, kind="ExternalInput").ap()
    out_d = nc.dram_tensor("out", [64, HI * W], F32, kind="ExternalOutput").ap()

    # gather source: quad entries of the padded image
    src_ap = bass.AP(xq_d, 0, [[256, NQ], [1, 256]])

    NM = K2 * HI            # 576 (k, i) pairs
    NFREE = NM * 8          # 4608 descriptors per 16-partition wrap

    with ExitStack() as ctx:
        tc = ctx.enter_context(tile.TileContext(nc))

        const = ctx.enter_context(tc.tile_pool(name="const", bufs=1))
        live = ctx.enter_context(tc.tile_pool(name="live", bufs=1))
        ph1 = ExitStack()
        work = ph1.enter_context(tc.tile_pool(name="work", bufs=1))
        ps1pool = ph1.enter_context(tc.tile_pool(name="ps1", bufs=2, space="PSUM"))

        offj = work.tile([128, 2 * K2 * HI], F32)
        nc.sync.dma_start(offj[:], offj_d)
        ejw = const.tile([128, 8 * 128], F32)
        nc.sync.dma_start(ejw[:], ejw_d)
        maskj = work.tile([128, K2 * HI], F32)
        nc.sync.dma_start(maskj[:], maskj_d)
        idxb = work.tile([128, NFREE], F32)
        nc.sync.dma_start(idxb[:], idxb_d)
        identb = const.tile([128, 128], BF16)
        nc.sync.dma_start(identb[:], identb_d)
        wk2 = const.tile([128, K2 * 128], BF16)
        nc.sync.dma_start(wk2[:], wk2_d)

        # warm the SWDGE gather ucode lib + first-call overhead while the
        # inputs load (gathers quad 0 = zeros; result unused)
        dummy_idx = const.tile([128, 1], I16)
        nc.gpsimd.memset(dummy_idx[:], 0)
        dummy_g = work.tile([128, 256], BF16)
        for q in range(4):
            nc.gpsimd.dma_gather(
                dummy_g[:].rearrange("p (s e) -> p s e", s=1, e=256), src_ap,
                dummy_idx[:], 16, 16, elem_size=256, queue_num=q)

        # ---- Phase 1a: floor, then dyx (critical path to the gathers) ----
        flo = work.tile([128, 2 * K2 * HI], F32)
        nc.vector.tensor_scalar(flo[:], offj[:], MAGIC, MAGIC, ALU.add,
                                ALU.subtract)
        rup = work.tile([128, 2 * K2 * HI], F32)
        nc.vector.tensor_tensor(rup[:], flo[:], offj[:], ALU.is_gt)
        nc.vector.tensor_tensor(flo[:], flo[:], rup[:], ALU.subtract)
        nc.vector.tensor_scalar(flo[:], flo[:], -CLAMP, CLAMP, ALU.max, ALU.min)

        def kv(t):  # [128, (k, two, i)]
            return t[:].rearrange("p (k two i) -> p k two i", k=K2, two=2, i=HI)

        # dyx[j, m=(k,i)] = floor(dy)*PW + floor(dx)
        dyx = work.tile([128, NM], F32)
        dyx3 = dyx[:].rearrange("p (k i) -> p k i", k=K2, i=HI)
        nc.vector.scalar_tensor_tensor(
            dyx3, kv(flo)[:, :, 0, :], float(PW), kv(flo)[:, :, 1, :],
            ALU.mult, ALU.add)

        # ---- Phase 1b: repack dyx [j, m] -> idxs [u-wrap, (m, jw)] -------
        # One selection matmul per jw: dp[p, m] = dyx[16*jw + p%16, m]
        # (stationary E_jw[j, p] = (j//16==jw) & (j%16==p%16)), then
        # idxs[p, m*8+jw] = idxb + dp with the add reading PSUM directly.
        frac = work.tile([128, 2 * K2 * HI], F32)
        nc.vector.tensor_tensor(frac[:], offj[:], flo[:], ALU.subtract)

        idxs = live.tile([128, NFREE], I16)
        idxs3 = idxs[:].rearrange("p (m jw) -> p m jw", m=NM, jw=8)
        idxb3 = idxb[:].rearrange("p (m jw) -> p m jw", m=NM, jw=8)
        dps = work.tile([128, 8 * NM], F32)
        for jp in range(4):            # two jw per PSUM tile
            dp = ps1pool.tile([128, 2 * NM], F32)
            for j2 in range(2):
                jw = jp * 2 + j2
                # chunk at psum-bank (512 f32) boundaries within the tile
                cuts = sorted({j2 * NM, (j2 + 1) * NM}
                              | {b for b in (512, 1024)
                                 if j2 * NM < b < (j2 + 1) * NM})
                for lo, hi in zip(cuts, cuts[1:]):
                    nc.tensor.matmul(
                        dp[:, lo:hi],
                        ejw[:, jw * 128:(jw + 1) * 128],
                        dyx[:, lo - j2 * NM:hi - j2 * NM],
                        start=True, stop=True)
            nc.scalar.copy(dps[:, jp * 2 * NM:(jp + 1) * 2 * NM], dp[:])
            nc.vector.tensor_tensor(
                idxs3[:, :, jp * 2:(jp + 1) * 2],
                idxb3[:, :, jp * 2:(jp + 1) * 2],
                dps[:, jp * 2 * NM:(jp + 1) * 2 * NM].rearrange(
                    "p (j2 m) -> p m j2", j2=2, m=NM),
                ALU.add)

        # ---- Phase 1c: corner weights w4[j, (k, i, q)] bf16, mask folded
        wy = kv(frac)[:, :, 0, :]      # [128, k, i]
        wx = kv(frac)[:, :, 1, :]
        omy = work.tile([128, NM], F32)
        omyv = omy[:].rearrange("p (k i) -> p k i", k=K2, i=HI)
        nc.vector.tensor_scalar(omyv, wy, 1.0, -1.0, ALU.subtract, ALU.mult)
        omx = work.tile([128, NM], F32)
        omxv = omx[:].rearrange("p (k i) -> p k i", k=K2, i=HI)
        nc.vector.tensor_scalar(omxv, wx, 1.0, -1.0, ALU.subtract, ALU.mult)
        m3 = maskj[:].rearrange("p (k i) -> p k i", k=K2, i=HI)
        wxm0 = work.tile([128, NM], F32)
        wxm0v = wxm0[:].rearrange("p (k i) -> p k i", k=K2, i=HI)
        nc.vector.tensor_tensor(wxm0v, omxv, m3, ALU.mult)
        wxm1 = work.tile([128, NM], F32)
        wxm1v = wxm1[:].rearrange("p (k i) -> p k i", k=K2, i=HI)
        nc.vector.tensor_tensor(wxm1v, wx, m3, ALU.mult)

        w4 = live.tile([128, NM * 4], BF16)
        w4v = w4[:].rearrange("p (k i q) -> p k i q", k=K2, i=HI, q=4)
        nc.vector.tensor_tensor(w4v[:, :, :, 0], omyv, wxm0v, ALU.mult)
        nc.vector.tensor_tensor(w4v[:, :, :, 1], omyv, wxm1v, ALU.mult)
        nc.vector.tensor_tensor(w4v[:, :, :, 2], wy, wxm0v, ALU.mult)
        nc.vector.tensor_tensor(w4v[:, :, :, 3], wy, wxm1v, ALU.mult)

        # ---- Phase 2: gather / combine / transpose / conv ----------------
        ph1.close()
        gpool = ctx.enter_context(tc.tile_pool(name="g", bufs=10))
        p4pool = ctx.enter_context(tc.tile_pool(name="p4", bufs=2))
        s2pool = ctx.enter_context(tc.tile_pool(name="s2", bufs=2))
        stpool = ctx.enter_context(tc.tile_pool(name="st", bufs=2))
        obpool = ctx.enter_context(tc.tile_pool(name="ob", bufs=2))
        tpps = ctx.enter_context(tc.tile_pool(name="tp", bufs=2, space="PSUM"))
        outps = ctx.enter_context(tc.tile_pool(name="ops", bufs=2, space="PSUM"))

        idxs4 = idxs[:].rearrange("p (k i jw) -> p k i jw", k=K2, i=HI, jw=8)
        w4r = w4[:].rearrange("p (k i q) -> p k i q", k=K2, i=HI, q=4)

        def idx_slice(k, r0, r1):
            return idxs4[:, k, r0:r1, :]

        with nc.allow_low_precision("bf16 deformable-conv pipeline"):
            for b in range(NBLK):
                out_ps = outps.tile([128, R * 64], F32)
                for k in range(K2):
                    g = gpool.tile([128, R * 256], BF16)
                    gv = g[:].rearrange("p (s e) -> p s e", s=R, e=256)
                    for sub in range(R // RSUB):
                        nc.gpsimd.dma_gather(
                            gv[:, sub * RSUB:(sub + 1) * RSUB, :], src_ap,
                            idx_slice(k, b * R + sub * RSUB,
                                      b * R + (sub + 1) * RSUB),
                            NIDX, NIDX, elem_size=256,
                            single_packet=False,
                            queue_num=(b * K2 * (R // RSUB) + k * (R // RSUB)
                                       + sub) % 4,
                        )
                    # weighted corners: p4 = g * w (w broadcast over c);
                    # y-corner sum pairwise (packed-pair reads keep DVE 2x);
                    # x-corner sum folds into the matmul (two accumulating
                    # planes e=0/1 share the same block-diag stationary).
                    # The very last (b, k) runs per 8-row half to shorten the
                    # post-last-gather tail.
                    p4 = p4pool.tile([128, R * 256], BF16)
                    s2 = s2pool.tile([128, R * C * 2], BF16)
                    tp = tpps.tile([128, 2 * 8 * 128], BF16)
                    st = stpool.tile([128, 2 * 8 * 128], BF16)
                    s2v = s2[:].rearrange("p (h x c e) -> p h x c e",
                                          h=R // 2, x=2, c=C, e=2)
                    for half in (0, 1):
                        if half is None:
                            r0, r1 = 0, R
                        else:
                            r0, r1 = half * RSUB, (half + 1) * RSUB
                        nr = r1 - r0
                        wsl = w4r[:, k, b * R + r0:b * R + r1, :]
                        w_b = bass.AP(
                            wsl.tensor, wsl.offset,
                            [wsl.ap[0], [4, nr], [0, C], [1, 4]],
                        )
                        nc.vector.tensor_tensor(
                            p4[:, r0 * 256:r1 * 256].rearrange(
                                "p (i c q) -> p i c q", i=nr, c=C, q=4),
                            g[:, r0 * 256:r1 * 256].rearrange(
                                "p (i c q) -> p i c q", i=nr, c=C, q=4),
                            w_b, ALU.mult)
                        p4q = p4[:, r0 * 256:r1 * 256].rearrange(
                            "p (ic q2 e) -> p ic q2 e", ic=nr * C, q2=2, e=2)
                        nc.vector.tensor_tensor(
                            s2[:, r0 * 128:r1 * 128].rearrange(
                                "p (ic e) -> p ic e", ic=nr * C, e=2),
                            p4q[:, :, 0, :], p4q[:, :, 1, :], ALU.add)
                        for e in range(2):
                            for h in range(r0 // 2, r1 // 2):
                                nc.tensor.transpose(
                                    tp[:, (e * 8 + h) * 128:
                                       (e * 8 + h + 1) * 128],
                                    s2v[:, h, :, :, e], identb[:])
                        nc.scalar.copy(
                            st[:].rearrange("p (e t) -> p e t",
                                            e=2, t=1024)[:, :, r0 * 64:r1 * 64],
                            tp[:].rearrange("p (e t) -> p e t",
                                            e=2, t=1024)[:, :, r0 * 64:r1 * 64])
                        for e in range(2):
                            for hh in range(r0 // 8, r1 // 8):
                                nc.tensor.matmul(
                                    out_ps[:, hh * 512:(hh + 1) * 512],
                                    wk2[:, k * 128:(k + 1) * 128],
                                    st[:, e * 1024 + hh * 512:
                                       e * 1024 + (hh + 1) * 512],
                                    start=(k == 0 and e == 0),
                                    stop=(k == K2 - 1 and e == 1))
                ob = obpool.tile([128, R * 64], F32)
                for half in (0, 1):
                    c0, c1 = (0, 1024) if half is None else (half * 512,
                                                            (half + 1) * 512)
                    nc.scalar.copy(ob[:, c0:c1], out_ps[:, c0:c1])
                    for i2 in range(2):
                        dst = bass.AP(
                            out_d.tensor,
                            out_d.offset + (b * R + c0 // 64 + i2) * W,
                            [out_d.ap[0], [2 * W, (c1 - c0) // 128], [1, W]],
                        )
                        nc.sync.dma_start(
                            dst,
                            ob[i2 * 64:(i2 + 1) * 64, c0:c1].rearrange(
                                "p (h j) -> p h j",
                                h=(c1 - c0) // 128, j=W))

    if not nc.is_finalized():
        nc.finalize()
    return nc


def _quad_image(xn):
    """xn: [C, H, W] f32 -> quad bf16 [NQ*256], entry (y,x) = 2x2 block,
    value order (c, q) with q = yc*2+xc."""
    xpad = np.zeros((PH + 1, PW + 1, C), dtype=BF)
    xpad[PAD:PAD + H, PAD:PAD + W, :] = xn.transpose(1, 2, 0).astype(BF)
    xq = np.empty((PH, PW, C, 4), dtype=BF)
    xq[:, :, :, 0] = xpad[0:PH, 0:PW]
    xq[:, :, :, 1] = xpad[0:PH, 1:PW + 1]
    xq[:, :, :, 2] = xpad[1:PH + 1, 0:PW]
    xq[:, :, :, 3] = xpad[1:PH + 1, 1:PW + 1]
    return np.ascontiguousarray(xq.reshape(-1))


def _static_prep(weight):
    # weight is [O, C_in, KH, KW]; reshape -> [O, C_in, K2]
    wk = weight.reshape(C, C, K2)
    wk2 = np.zeros((128, K2, 128), np.float32)
    for i2 in range(2):
        # rows (i2*64 + c), cols (i2*64 + o) = W[o, c, k]
        wk2[i2 * 64:(i2 + 1) * 64, :, i2 * 64:(i2 + 1) * 64] = (
            wk.transpose(1, 2, 0))
    return wk2.astype(BF).reshape(128, K2 * 128)


def _prep_core(x, offset, mask, wk2, xq_cache, core):
    n, half = core // 2, core % 2
    i0 = half * HI
    if n not in xq_cache:
        xq_cache[n] = _quad_image(x[n])
    offj = np.ascontiguousarray(
        offset[n, :, i0:i0 + HI, :].transpose(2, 0, 1)).reshape(128, 2 * K2 * HI)
    maskj = np.ascontiguousarray(
        mask[n, :, i0:i0 + HI, :].transpose(2, 0, 1)).reshape(128, K2 * HI)

    u = np.arange(128) % 16
    k = np.arange(K2)
    ki, kj = k // 3, k % 3
    i = np.arange(HI)
    jw = np.arange(8)
    # idxb[u, (k, i, jw)] = (PAD+i0+i+ki-1)*PW + PAD + jw*16 + u + kj - 1
    base = ((PAD + i0 + i[None, :, None] + ki[:, None, None] - 1) * PW
            + PAD + jw[None, None, :] * 16 + kj[:, None, None] - 1)  # [k, i, jw]
    idxb = (base[None] + u[:, None, None, None]).reshape(128, -1)
    assert idxb.min() - CLAMP * PW - CLAMP >= 0
    assert idxb.max() + CLAMP * PW + CLAMP < NQ

    jj = np.arange(128)
    pp = np.arange(128)
    ejw = ((jj[:, None] % 16 == pp[None, :] % 16)[:, None, :]
           & (jj[:, None, None] // 16 == np.arange(8)[None, :, None])
           ).astype(np.float32).reshape(128, 8 * 128)

    return {
        "xq": xq_cache[n],
        "offj": offj,
        "maskj": maskj,
        "idxb": idxb.astype(np.float32),
        "wk2": wk2,
        "ejw": ejw,
        "identb": np.eye(128, dtype=BF),
    }


def _prep_all(x, offset, mask, weight):
    x = np.asarray(x, np.float32)
    offset = np.asarray(offset, np.float32)
    mask = np.asarray(mask, np.float32)
    weight = np.asarray(weight, np.float32)
    wk2 = _static_prep(weight)
    xq_cache = {}
    return [
        _prep_core(x, offset, mask, wk2, xq_cache, core) for core in range(8)
    ]


def _collect(res):
    out = np.empty((N, C, H, W), np.float32)
    for core in range(8):
        n, half = core // 2, core % 2
        out[n, :, half * HI:(half + 1) * HI, :] = (
            res.results[core]["out"].reshape(C, HI, W))
    return out


def kernel_traced(x, offset, mask, weight, trace=True, trace_kwargs=None):
    """Like kernel() but with NTFF tracing; returns (out, BassKernelResults)."""
    if "nc" not in _CACHED:
        _CACHED["nc"] = build_nc()
    in_maps = _prep_all(x, offset, mask, weight)
    res = run_bass_kernel_spmd(_CACHED["nc"], in_maps, list(range(8)),
                               trace=trace, **(trace_kwargs or {}))
    return _collect(res), res


def kernel(x, offset, mask, weight):
    if "nc" not in _CACHED:
        _CACHED["nc"] = build_nc()
    in_maps = _prep_all(x, offset, mask, weight)
    res = run_bass_kernel_spmd(_CACHED["nc"], in_maps, list(range(8)))
    return _collect(res)
